# revision 24
# baseline (speedup 1.0000x reference)
"""Trainium2 kernel for CompactBilinearLayer (count-sketch bilinear pooling).

Math: y = l2norm(signed_sqrt(sum_hw Re IFFT(FFT(x@M1)*FFT(x@M2)))).
FFT(x@M1) == x @ A1 with A1[c,k] = s1[c] exp(-2pi i h1[c] k/P) (dense [C,K],
host-built).  IFFT is linear so the spatial sum moves before it; Hermitian
symmetry keeps only k = 0..4096 (padded to 33*128 slots).

Per core (4 batch elems, T=784 spatial positions, no collectives):
  A: P-planes = A^T @ x^T in bf16 (tolerance 2e-2 >> bf16 error) as two
     2-plane PSUM super-tiles (re1,im1) and (re2,-im2).
  B: casts to bf16 SBUF (with an extra negated im2 plane so both complex
     product groups are pure ADDs), pair-packed DVE products, bf16 pair-fold,
     one segmented reduce -> S[k, b] (re, im).
  C: per kt twiddle U=cphi*Sre-sphi*Sim, V=sphi*Sre+cphi*Sim as packed
     TTs (GpSimd+DVE), accumulated over kt into Utot/Vtot; since the DFT-128
     matrix depends only on k mod 128, IFFT = 2 matmuls at the end.
  D: signed sqrt + per-batch L2 norm + store.
"""
import numpy as np

P = 8192
C = 512
FT = 33            # frequency tiles of 128 -> 4224 slots >= 4097
NCORES = 8
BPC = 4            # batch elems per core
HW = 196           # spatial positions per batch elem
T = BPC * HW       # 784 positions per core
B = 32

_CACHE = {}


def _build_program():
    import concourse.bass as bass
    import concourse.tile as tile
    from concourse import bacc, mybir

    f32 = mybir.dt.float32
    f16 = mybir.dt.float16
    nc = bacc.Bacc("TRN2", target_bir_lowering=False, debug=False,
                   num_devices=NCORES)

    a_d = nc.dram_tensor("a", [FT, C, 512], f16, kind="ExternalInput").ap()
    x_d = nc.dram_tensor("x", [C, T], f16, kind="ExternalInput").ap()
    cucv_d = nc.dram_tensor("cucv", [FT, 128, 4, 64], f32,
                            kind="ExternalInput").ap()
    cosa_d = nc.dram_tensor("cosa", [128, 128], f32, kind="ExternalInput").ap()
    nsina_d = nc.dram_tensor("nsina", [128, 128], f32, kind="ExternalInput").ap()
    y_d = nc.dram_tensor("y", [BPC, P], f32, kind="ExternalOutput").ap()

    mult = mybir.AluOpType.mult
    add = mybir.AluOpType.add
    Act = mybir.ActivationFunctionType

    with tile.TileContext(nc) as tc:
        with (
            tc.tile_pool(name="const", bufs=1) as const,
            tc.tile_pool(name="apool", bufs=3) as apool,
            tc.tile_pool(name="ps", bufs=1, space="PSUM") as pspool,
            tc.tile_pool(name="cast", bufs=3) as castp,
            tc.tile_pool(name="gp", bufs=3) as gpool,
            tc.tile_pool(name="uv", bufs=3) as uvpool,
            tc.tile_pool(name="sf", bufs=3) as sfpool,
            tc.tile_pool(name="scr", bufs=2) as scr,
        ):
            x_sb = const.tile([128, 4, T], f16)
            nc.sync.dma_start(x_sb[:], x_d.rearrange("(ck p) t -> p ck t", p=128))
            a_pre = {}
            for ft in (0, 1, 2):
                a_pre[ft] = apool.tile([128, 4, 512], f16, tag="a",
                                       name=f"a_pre{ft}")
                nc.sync.dma_start(
                    a_pre[ft][:], a_d[ft].rearrange("(ck p) m -> p ck m", p=128)
                )
            cucv_sb = const.tile([128, FT, 4, 64], f32)
            nc.sync.dma_start(cucv_sb[:], cucv_d.rearrange("kt p c s -> p kt c s"))
            cosa_sb = const.tile([128, 128], f32)
            nc.sync.dma_start(cosa_sb[:], cosa_d)
            nsina_sb = const.tile([128, 128], f32)
            nc.sync.dma_start(nsina_sb[:], nsina_d)
            ones_sb = const.tile([128, 1], f32)
            nc.vector.memset(ones_sb[:], 1.0)
            onesrow = const.tile([1, 128], f32)
            nc.vector.memset(onesrow[:], 1.0)

            # U/V accumulators [u1, v1, u2n, v2]: chain A on DVE (kt%6==5),
            # chain B on GpSimd (the rest); each ping-pongs two buffers.
            accs = {}
            for name in ("a0", "a1", "b0", "b1"):
                accs[name] = const.tile([128, 4, BPC * 64], f32,
                                        tag=f"acc_{name}",
                                        name=f"acc_{name}")
            nc.vector.memset(accs["a0"][:], 0.0)
            nc.gpsimd.memset(accs["b0"][:], 0.0)
            na = nb = 0   # adds done per chain

            for ft in range(FT):
                if ft in a_pre:
                    a_t = a_pre.pop(ft)
                else:
                    a_t = apool.tile([128, 4, 512], f16, tag="a",
                                     name=f"a_{ft}")
                    nc.sync.dma_start(
                        a_t[:], a_d[ft].rearrange("(ck p) m -> p ck m", p=128)
                    )
                # two 2-plane PSUM super-tiles, plane stride 1024 (2 banks)
                ps1 = pspool.tile([128, 2, 1024], f32, tag="p1", name=f"ps1_{ft}")
                ps2 = pspool.tile([128, 2, 1024], f32, tag="p2", name=f"ps2_{ft}")
                for half, pst in ((0, ps1), (1, ps2)):
                    for pl in range(2):
                        m = half * 2 + pl
                        msl = slice(m * 128, (m + 1) * 128)
                        for c0, cn in ((0, 512), (512, T - 512)):
                            for ck in range(4):
                                nc.tensor.matmul(
                                    pst[:, pl, c0:c0 + cn],
                                    a_t[:, ck, msl],
                                    x_sb[:, ck, c0:c0 + cn],
                                    start=(ck == 0),
                                    stop=(ck == 3),
                                )
                # casts: c1 = [re1, im1]; c2x = [im2, re2, im2n]
                c1 = castp.tile([128, 2, T], f16, tag="c1", name=f"c1_{ft}")
                nc.scalar.activation(c1[:], ps1[:, :, 0:T], Act.Copy)
                c2x = castp.tile([128, 3, T], f16, tag="c2", name=f"c2_{ft}")
                nc.scalar.activation(c2x[:, 1:3, :], ps2[:, :, 0:T], Act.Copy)
                nc.scalar.activation(c2x[:, 0, :], ps2[:, 1, 0:T], Act.Copy,
                                     scale=-1.0)
                # products: G[g, pl, t]; g0 = [rere, -imim], g1 = [reim, imre]
                G = gpool.tile([128, 2, 2, T], f16, tag="G", name=f"G_{ft}")
                nc.vector.tensor_tensor(G[:, 0], c1[:], c2x[:, 1:3, :], op=mult)
                nc.vector.tensor_tensor(G[:, 1], c1[:], c2x[:, 0:2, :], op=mult)
                # one fused segmented reduce over t (the (g,pl,b) dims merge:
                # strides are perfectly nested), then fold the pl pairs
                r16 = sfpool.tile([128, 16], f16, tag="r16", name=f"r16_{ft}")
                with nc.allow_low_precision("fp16 partial sums ok at 2e-2 tol"):
                    nc.vector.reduce_sum(
                        out=r16[:],
                        in_=G[:].rearrange("p g l (b t) -> p (g l b) t", b=BPC),
                        axis=mybir.AxisListType.X,
                    )
                sf = sfpool.tile([128, 2, BPC], f32, tag="sf", name=f"sf_{ft}")
                r4 = r16[:].rearrange("p (g l b) -> p g l b", g=2, l=2)
                nc.gpsimd.tensor_tensor(sf[:], r4[:, :, 0], r4[:, :, 1], op=add)

                # ---- stage C: W = [u1, v1, u2n, v2] = cucv * S, accumulate
                kt = ft
                sre_b = sf[:, 0:1, :][:, :, :, None].broadcast_to(
                    [128, 2, BPC, 64])
                sim_b = sf[:, 1:2, :][:, :, :, None].broadcast_to(
                    [128, 2, BPC, 64])
                cc1 = cucv_sb[:, kt, 0:2][:, :, None, :].broadcast_to(
                    [128, 2, BPC, 64])
                cc2 = cucv_sb[:, kt, 2:4][:, :, None, :].broadcast_to(
                    [128, 2, BPC, 64])
                W = uvpool.tile([128, 4, BPC * 64], f32, tag="W", name=f"W_{kt}")
                w4 = W[:].rearrange("p c (b s) -> p c b s", s=64)
                nc.gpsimd.tensor_tensor(w4[:, 0:2], cc1, sre_b, op=mult)
                nc.gpsimd.tensor_tensor(w4[:, 2:4], cc2, sim_b, op=mult)
                if kt % 6 == 5:
                    src_t = accs["a0"] if na % 2 == 0 else accs["a1"]
                    dst_t = accs["a1"] if na % 2 == 0 else accs["a0"]
                    nc.gpsimd.tensor_tensor(dst_t[:], src_t[:], W[:], op=add)
                    na += 1
                else:
                    src_t = accs["b0"] if nb % 2 == 0 else accs["b1"]
                    dst_t = accs["b1"] if nb % 2 == 0 else accs["b0"]
                    nc.gpsimd.tensor_tensor(dst_t[:], src_t[:], W[:], op=add)
                    nb += 1

            warm = scr.tile([1, 1], f32, tag="warm")
            nc.scalar.activation(warm[:], ones_sb[0:1, :], Act.Sqrt)

            accA = accs["a1"] if na % 2 == 1 else accs["a0"]
            accB = accs["b1"] if nb % 2 == 1 else accs["b0"]

            # ---- IFFT over k mod 128: psy = cosa@Utot + nsina@Vtot,
            # accumulating both chains' comps directly in PSUM
            psy_t = pspool.tile([128, 2, 1024], f32, tag="p1", name="psy_t")
            psy = psy_t[:, 0, 0:BPC * 64]
            mats = (cosa_sb, nsina_sb, cosa_sb, nsina_sb)
            first = True
            for chain in (accA, accB):
                for comp in range(4):
                    nc.tensor.matmul(psy, mats[comp][:], chain[:, comp, :],
                                     start=first,
                                     stop=(chain is accB and comp == 3))
                    first = False

            # ---- stage D: signed sqrt, per-batch l2 norm, store ----
            absy = scr.tile([128, BPC * 64], f32, tag="absy")
            nc.scalar.activation(absy[:], psy, Act.Abs)
            sqy = scr.tile([128, BPC * 64], f32, tag="sqy")
            nc.scalar.activation(sqy[:], absy[:], Act.Sqrt)
            sgn = scr.tile([128, BPC * 64], f32, tag="sgn")
            nc.scalar.activation(sgn[:], psy, Act.Sign)
            ys = scr.tile([128, BPC * 64], f32, tag="ys")
            nc.vector.tensor_mul(ys[:], sqy[:], sgn[:])

            psn_t = pspool.tile([128, 2, 1024], f32, tag="p2", name="psn_t")
            psn = psn_t[:, 0, 0:BPC * 64]
            nc.tensor.matmul(psn[0:1, :], ones_sb[:], absy[:],
                             start=True, stop=True)
            nsq = scr.tile([1, BPC], f32, tag="nsq")
            nc.vector.reduce_sum(
                out=nsq[:],
                in_=psn[0:1, :].rearrange("p (b s) -> p b s", b=BPC),
                axis=mybir.AxisListType.X,
            )
            nc.vector.tensor_scalar_max(nsq[:], nsq[:], 1e-10)
            sqn = scr.tile([1, BPC], f32, tag="sqn")
            nc.scalar.activation(sqn[:], nsq[:], Act.Sqrt)
            invn = scr.tile([1, BPC], f32, tag="invn")
            nc.vector.reciprocal(invn[:], sqn[:])

            psb_t = pspool.tile([128, 2, 1024], f32, tag="p1", name="psb_t")
            psb = psb_t[:, 1, 0:BPC * 64]
            nc.tensor.matmul(psb[:, 0:BPC], onesrow[0:1, :], invn[0:1, :],
                             start=True, stop=True)
            inv_b = psb[:, 0:BPC][:, :, None].broadcast_to([128, BPC, 64])
            fin = scr.tile([128, BPC * 64], f32, tag="fin")
            nc.vector.tensor_tensor(
                fin[:].rearrange("p (b s) -> p b s", b=BPC),
                ys[:].rearrange("p (b s) -> p b s", b=BPC),
                inv_b,
                op=mult,
            )
            nc.sync.dma_start(
                y_d.rearrange("b (q s) -> q b s", q=128),
                fin[:].rearrange("p (b s) -> p b s", b=BPC),
            )

    nc.compile()
    return nc


def _host_prep(x, M1, M2):
    x = np.ascontiguousarray(np.asarray(x, np.float32))
    M1 = np.asarray(M1, np.float32)
    M2 = np.asarray(M2, np.float32)

    h1 = np.argmax(np.abs(M1), axis=1)
    s1 = M1[np.arange(C), h1].astype(np.float64)
    h2 = np.argmax(np.abs(M2), axis=1)
    s2 = M2[np.arange(C), h2].astype(np.float64)

    NSLOT = FT * 128
    k = np.arange(NSLOT, dtype=np.float64)
    valid = k <= P // 2
    ang1 = 2 * np.pi * np.outer(h1.astype(np.float64), k) / P
    ang2 = 2 * np.pi * np.outer(h2.astype(np.float64), k) / P
    # a[ft, c, m*128 + j]: m in (A1re, A1im, A2re, -A2im), freq = ft*128 + j
    a = np.empty((FT, C, 512), np.float32)
    a1re = (s1[:, None] * np.cos(ang1) * valid).astype(np.float32)
    a1im = (-s1[:, None] * np.sin(ang1) * valid).astype(np.float32)
    a2re = (s2[:, None] * np.cos(ang2) * valid).astype(np.float32)
    a2imn = (s2[:, None] * np.sin(ang2) * valid).astype(np.float32)  # -A2im
    for ft in range(FT):
        ksl = slice(ft * 128, (ft + 1) * 128)
        a[ft, :, 0:128] = a1re[:, ksl]
        a[ft, :, 128:256] = a1im[:, ksl]
        a[ft, :, 256:384] = a2re[:, ksl]
        a[ft, :, 384:512] = a2imn[:, ksl]

    w = np.where(valid, 2.0 / P, 0.0)
    w[0] = 1.0 / P
    w[P // 2] = 1.0 / P
    s_idx = np.arange(64, dtype=np.float64)
    phi = 2 * np.pi * np.outer(k, s_idx) / P
    cphi = (w[:, None] * np.cos(phi)).astype(np.float32).reshape(FT, 128, 64)
    sphi = (w[:, None] * np.sin(phi)).astype(np.float32).reshape(FT, 128, 64)
    # cucv comps: [cphi, sphi, -sphi, cphi] so W = [u1, v1, u2n, v2]
    cucv = np.stack([cphi, sphi, -sphi, cphi], axis=2)  # [FT, 128, 4, 64]

    km = np.arange(128, dtype=np.float64)
    alpha = 2 * np.pi * np.outer(km, km) / 128
    cosa = np.cos(alpha).astype(np.float32)
    nsina = (-np.sin(alpha)).astype(np.float32)

    xt = np.ascontiguousarray(x.reshape(B * HW, C).T)  # [C, 6272]

    return (a.astype(np.float16), cucv, cosa, nsina, xt.astype(np.float16))


def _make_in_maps(x, M1, M2):
    a, cucv, cosa, nsina, xt = _host_prep(x, M1, M2)
    in_maps = []
    for r in range(NCORES):
        in_maps.append({
            "a": a,
            "x": np.ascontiguousarray(xt[:, r * T:(r + 1) * T]),
            "cucv": cucv,
            "cosa": cosa,
            "nsina": nsina,
        })
    return in_maps


def kernel(x, M1, M2):
    from concourse.bass_utils import run_bass_kernel_spmd

    if "nc" not in _CACHE:
        _CACHE["nc"] = _build_program()
    nc = _CACHE["nc"]

    in_maps = _make_in_maps(x, M1, M2)
    res = run_bass_kernel_spmd(nc, in_maps, core_ids=list(range(NCORES)))
    out = np.concatenate([res.results[r]["y"] for r in range(NCORES)], axis=0)
    return out.astype(np.float32)
